# revision 1
# baseline (speedup 1.0000x reference)
"""Trainium2 Bass kernel for nn_BestModel5 (dual-GRU encoder + BxB pair classifier).

Sharding (8 cores): cores 0-3 query-GRU batch shards of 64; cores 4-7 reply-GRU.
Classifier sharded 8-way over the 256 query rows (32 i-rows/core).
Embedding gather + input projection layout prep on host; all matmuls bf16 on PE,
f32 PSUM accumulate, nonlinearities f32 on ACT.

Encoding exchange uses the NRT mesh AllGather (~38us for 32KB, mostly fixed
overhead). A SBUF->SBUF remote_dma_broadcast path exists behind
KERNEL_USE_RDMA=1 but measured SLOWER (~50us: SWDGE per-descriptor cost
dominates at this payload; 14 of 16 descs per broadcast are dummies).
GRU step: r/z gate MMs in separate PSUM banks (r chain starts after 4 MMs),
z1=sig(gz) and z2=sig(-gz) so the post-tanh path is mul+add (no subtract);
b2 bias and the classifier column un-permute are applied on the host.
"""

import os

import numpy as np
import ml_dtypes

BF16 = ml_dtypes.bfloat16

def _enable_ldw_opt():
    """LDWEIGHTS pipelining is disabled in this environment's default
    compiler flags; re-enable it (measured ~90us of serial weight loads)."""
    from concourse.compiler_utils import get_compiler_flags, set_compiler_flags

    flags = [f.replace("--enable-ldw-opt=false", "--enable-ldw-opt=true")
             for f in get_compiler_flags()]
    set_compiler_flags(flags)

V, E, H, B, T = 100000, 256, 256, 256, 40
D_HID, D_OUT = 256, 2
NCORES = 8
BSH = 64          # batch rows per GRU shard
NSH = 4           # GRU batch shards per encoder
BT = BSH * T      # 2560 columns of XembT per core
IBLK = B // NCORES  # 32 classifier i-rows per core

USE_RDMA = os.environ.get("KERNEL_USE_RDMA", "0") == "1"

# remote_dma_broadcast delivery map: slot j on core r holds core (r^F[j])'s
# tile (bit-2 slots get an extra XOR of 2 from the D2D hop).
F_SLOT = [0, 1, 2, 3, 6, 7, 4, 5]


def _jblock(core, s):
    """Reply block (0-3) sitting at rT slot-position s on this core."""
    if not USE_RDMA:
        return s
    if core < 4:
        return (s ^ 2) ^ core
    return (core - 4) ^ s


def _jperm(core):
    """Device column j_local -> global reply index, per core."""
    import numpy as _np
    return _np.concatenate(
        [64 * _jblock(core, s) + _np.arange(64) for s in range(4)])

_cache = {}


def _build(sim_gelu=False):
    """Build + compile the SPMD Bass program once. Returns (nc, out_name)."""
    import concourse.bacc as bacc
    import concourse.tile as tile
    import concourse.mybir as mybir

    if os.environ.get("KERNEL_LDW_OPT", "1") == "1":
        _enable_ldw_opt()

    f32 = mybir.dt.float32
    bf16 = mybir.dt.bfloat16
    AF = mybir.ActivationFunctionType

    nc = bacc.Bacc("TRN2", target_bir_lowering=False, debug=False, num_devices=NCORES)

    def din(name, shape, dt):
        return nc.dram_tensor(name, shape, dt, kind="ExternalInput").ap()

    # per-core inputs (content differs per core; shapes identical)
    xembT = din("xembT", [E + 2, BT], bf16)      # rows 0-255 emb, 256 mask, 257 ones
    whg = din("whg", [H, 2 * H], bf16)           # Wg[E:E+H, :]
    wxg = din("wxg", [E + 2, 2 * H], bf16)       # Wg[:E, :] + mask row (30 on z cols) + bg row
    wch = din("wch", [H, H], bf16)               # Wc[E:E+H, :]
    wxc = din("wxc", [E + 2, H], bf16)           # Wc[:E, :] + zero row + bc row
    w1q = din("w1q", [H, D_HID], bf16)           # W1[:256]
    w1r = din("w1r", [H, D_HID], bf16)           # W1[257:513]
    wdt = din("wdt", [1, IBLK // 2 * D_HID], bf16)  # W1[256] tiled 16x
    rhsb = din("rhsb", [4, IBLK * B], bf16)      # [0;ones|0;0|0;0;ones] pattern
    b1 = din("b1", [D_HID], f32)
    w2 = din("w2", [D_HID, D_OUT], bf16)

    out = nc.dram_tensor("out", [D_OUT, IBLK * B], f32, kind="ExternalOutput").ap()

    with tile.TileContext(nc) as tc:
        with (
            tc.tile_pool(name="persist", bufs=1) as pp,
            tc.tile_pool(name="dram", bufs=1, space="DRAM") as dramp,
        ):
            # ---- load weights/constants to SBUF ----
            xT = [pp.tile([128, BT], bf16, tag=f"xT{k}", name=f"xT{k}") for k in range(2)]
            xTm = pp.tile([2, BT], bf16, tag="xTm", name="xTm")
            # gate x-weights first on sync: step 0's x-matmuls gate on them
            # (each DMA dispatch is ~650ns serial per queue)
            wxg_s = [pp.tile([128, 2 * H], bf16, tag=f"wxg{k}", name=f"wxg{k}") for k in range(2)]
            wxg_m = pp.tile([2, 2 * H], bf16, tag="wxgm", name="wxgm")
            nc.sync.dma_start(wxg_m[:], wxg[256:258, :])
            nc.sync.dma_start(wxg_s[0][:], wxg[0:128, :])
            nc.sync.dma_start(wxg_s[1][:], wxg[128:256, :])
            dma_eng = [nc.sync, nc.scalar, nc.gpsimd]
            for n in range(5):
                cs = slice(512 * n, 512 * n + 512)
                dma_eng[n % 3].dma_start(xT[0][:, cs], xembT[0:128, cs])
                dma_eng[(n + 1) % 3].dma_start(xT[1][:, cs], xembT[128:256, cs])
                dma_eng[(n + 2) % 3].dma_start(xTm[:, cs], xembT[256:258, cs])

            # warm the ACT table set covering sigmoid+tanh+gelu during the
            # DMA phase: the lazy load otherwise costs 1.3us right before
            # step 0's first sigmoid. Must include Gelu - warming only
            # sigmoid/tanh selects a smaller set and pushes a second table
            # load onto the classifier start.
            warm = pp.tile([1, 3], f32, tag="warm", name="warm")
            nc.scalar.activation(warm[:, 0:1], xTm[0:1, 0:1], AF.Sigmoid)
            nc.scalar.activation(warm[:, 1:2], xTm[0:1, 0:1], AF.Tanh)
            nc.scalar.activation(warm[:, 2:3], xTm[0:1, 0:1],
                                 AF.Tanh if sim_gelu else AF.Gelu_apprx_tanh)
            warm_d = dramp.tile([1, 3], f32, tag="warmd", name="warmd")
            nc.scalar.dma_start(warm_d[:], warm[:])
            wxc_s = [pp.tile([128, H], bf16, tag=f"wxc{k}", name=f"wxc{k}") for k in range(2)]
            nc.sync.dma_start(wxc_s[0][:], wxc[0:128, :])
            nc.sync.dma_start(wxc_s[1][:], wxc[128:256, :])
            wxc_b = pp.tile([2, H], bf16, tag="wxcb", name="wxcb")
            nc.sync.dma_start(wxc_b[:], wxc[256:258, :])

            whg_s = [pp.tile([128, 2 * H], bf16, tag=f"whg{k}", name=f"whg{k}") for k in range(2)]
            nc.gpsimd.dma_start(whg_s[0][:], whg[0:128, :])
            nc.gpsimd.dma_start(whg_s[1][:], whg[128:256, :])
            wch_s = [pp.tile([128, H], bf16, tag=f"wch{k}", name=f"wch{k}") for k in range(2)]
            nc.gpsimd.dma_start(wch_s[0][:], wch[0:128, :])
            nc.gpsimd.dma_start(wch_s[1][:], wch[128:256, :])

            w1q_s = [pp.tile([128, D_HID], bf16, tag=f"w1q{k}", name=f"w1q{k}") for k in range(2)]
            nc.scalar.dma_start(w1q_s[0][:], w1q[0:128, :])
            nc.scalar.dma_start(w1q_s[1][:], w1q[128:256, :])
            w1r_s = [pp.tile([128, D_HID], bf16, tag=f"w1r{k}", name=f"w1r{k}") for k in range(2)]
            nc.scalar.dma_start(w1r_s[0][:], w1r[0:128, :])
            nc.scalar.dma_start(w1r_s[1][:], w1r[128:256, :])
            b1_s = pp.tile([128, 2], f32, tag="b1", name="b1")
            nc.scalar.dma_start(b1_s[:], b1.rearrange("(m p) -> p m", p=128))
            w2_s = [pp.tile([128, D_OUT], bf16, tag=f"w2{k}", name=f"w2{k}") for k in range(2)]
            nc.scalar.dma_start(w2_s[0][:], w2[0:128, :])
            nc.scalar.dma_start(w2_s[1][:], w2[128:256, :])


            if USE_RDMA:
                # exchange buffers + early desc-gen: descs only reference pk's
                # address (its data is read at trigger time, after the GRU),
                # so the SWDGE lib swap + desc writes hide behind the
                # precompute phase. gpsimd must run no other Q7-library ops
                # between here and the trigger.
                pk = pp.tile([16, NCORES * 128], bf16, tag="pk", name="pk")
                ag_p = pp.tile([16, NCORES * 1024], bf16, tag="agp",
                               name="agp")
                rsem = nc.alloc_semaphore("xchg_remote")
                lsem = nc.alloc_semaphore("xchg_local")
                psem = nc.alloc_semaphore("xchg_prep")
                for j in range(NCORES):
                    rdests = [None] * NCORES
                    rdests[j] = (0, j)
                    nc.gpsimd.remote_dma_broadcast(
                        ag_p[:, 1024 * j:1024 * (j + 1)], pk[:],
                        remote_sem=rsem, local_sem=lsem,
                        rdests=rdests).then_inc(psem, 1)

            # ---- GRU recurrence: x-projections fused into each step ----
            # The x-part matmuls depend only on the static xT tiles, so they
            # are emitted ahead of the h-part and execute in PE idle time
            # while the previous step's elementwise tail runs. Bias and
            # length-mask ride along as extra lhsT rows against the mask/ones
            # rows of xembT, so sigmoid/tanh read the PSUM preactivations
            # directly: no vector adds on the critical path, no precompute
            # phase. x-groups close (stop=True) before the h-part reopens the
            # region with start=False -- PE-only, strictly sequential groups.
            with (
                tc.tile_pool(name="gpsum", bufs=2, space="PSUM") as gps,
                tc.tile_pool(name="zpsum", bufs=1, space="PSUM") as zps,
                tc.tile_pool(name="cpsum", bufs=2, space="PSUM") as cps,
                tc.tile_pool(name="cpsum1", bufs=1, space="PSUM") as cps1,
                tc.tile_pool(name="step", bufs=2) as sp,
            ):
                h_bf = [pp.tile([128, 64], bf16, tag=f"hbf{k}",
                                name=f"hbf{k}", bufs=2) for k in range(2)]
                h_f = pp.tile([128, 128], f32, tag="hf", name="hf", bufs=2)
                nc.vector.memset(h_bf[0][:], 0.0)
                nc.vector.memset(h_bf[1][:], 0.0)
                nc.vector.memset(h_f[:], 0.0)

                for t in range(T):
                    ts = slice(64 * t, 64 * t + 64)
                    # one open accumulation group per bank region: the x-part
                    # opens it (start=True, no h dependency -> runs in PE idle
                    # time), the h-part closes it. Groups in different banks
                    # interleave freely; within a bank regions are strictly
                    # sequential (reopen-after-stop corrupts PSUM).
                    g_r = [gps.tile([128, 64], f32, tag=f"gr{m}",
                                    name=f"gr{m}") for m in range(2)]
                    g_z = zps.tile([128, 128], f32, tag="gpz", name="gpz")
                    c_ps = [cps.tile([128, 64], f32, tag="cps0", name="cps0"),
                            cps1.tile([128, 64], f32, tag="cps1",
                                      name="cps1")]

                    def xpart(ps_ap, wx, wmb, mm, stop=False):
                        for k in range(2):
                            nc.tensor.matmul(
                                ps_ap, wx[k][:, 128 * mm:128 * mm + 128],
                                xT[k][:, ts], start=(k == 0), stop=False,
                                skip_group_check=True)
                        nc.tensor.matmul(
                            ps_ap, wmb[:, 128 * mm:128 * mm + 128],
                            xTm[:, ts], start=False, stop=stop,
                            skip_group_check=True)

                    def hpart(ps_ap, mm, rhs):
                        for k in range(2):
                            nc.tensor.matmul(
                                ps_ap, whg_s[k][:, 128 * mm:128 * mm + 128],
                                rhs[k][:], start=False, stop=(k == 1),
                                skip_group_check=True)

                    # dependency-free x-projections first
                    for m in range(2):
                        xpart(g_r[m][:], wxg_s, wxg_m, m)
                    xpart(g_z[:, 0:64], wxg_s, wxg_m, 2)
                    xpart(c_ps[0][:], wxc_s, wxc_b, 0)
                    xpart(c_ps[1][:], wxc_s, wxc_b, 1)
                    # h-parts close the r and z(m0) groups once h arrives
                    for m in range(2):
                        hpart(g_r[m][:], m, h_bf)
                    hpart(g_z[:, 0:64], 2, h_bf)
                    xpart(g_z[:, 64:128], wxg_s, wxg_m, 3)
                    hpart(g_z[:, 64:128], 3, h_bf)

                    sig_r = [sp.tile([128, 64], f32, tag=f"sig{k}",
                                     name=f"sig{k}") for k in range(2)]
                    rh_bf = [sp.tile([128, 64], bf16, tag=f"rh{k}",
                                     name=f"rh{k}") for k in range(2)]
                    for k in range(2):
                        nc.scalar.activation(sig_r[k][:], g_r[k][:],
                                             AF.Sigmoid)
                        nc.vector.tensor_mul(rh_bf[k][:], sig_r[k][:],
                                             h_f[:, 64 * k:64 * k + 64])
                    # candidate h-parts close each half-bank group; tanh of
                    # half m fires as soon as its 2 h-MMs land
                    for m in range(2):
                        for k in range(2):
                            nc.tensor.matmul(
                                c_ps[m][:], wch_s[k][:, 128 * m:128 * m + 128],
                                rh_bf[k][:], start=False, stop=(k == 1),
                                skip_group_check=True)
                    # z path off the critical chain: z2 = sig(-gz) = 1-z1;
                    # h' = z1*h + z2*c needs only mul+add after tanh
                    z2 = sp.tile([128, 128], f32, tag="z2", name="z2")
                    nc.scalar.activation(z2[:], g_z[:], AF.Sigmoid, scale=-1.0)
                    z1 = sp.tile([128, 128], f32, tag="z1", name="z1")
                    nc.scalar.activation(z1[:], g_z[:], AF.Sigmoid)
                    m1 = sp.tile([128, 128], f32, tag="m1", name="m1")
                    nc.gpsimd.tensor_mul(m1[:], z1[:], h_f[:])
                    c_t = [sp.tile([128, 64], f32, tag=f"ct{k}",
                                   name=f"ct{k}") for k in range(2)]
                    z2c = [sp.tile([128, 64], f32, tag=f"z2c{k}",
                                   name=f"z2c{k}") for k in range(2)]
                    h_bf = [pp.tile([128, 64], bf16, tag=f"hbf{k}",
                                    name=f"hbf{k}", bufs=2) for k in range(2)]
                    for k in range(2):
                        ks = slice(64 * k, 64 * k + 64)
                        nc.scalar.activation(c_t[k][:], c_ps[k][:], AF.Tanh)
                        nc.vector.tensor_mul(z2c[k][:], z2[:, ks], c_t[k][:])
                        nc.vector.tensor_add(h_bf[k][:], m1[:, ks], z2c[k][:])
                    h_f_new = pp.tile([128, 128], f32, tag="hf", name="hf", bufs=2)
                    for k in range(2):
                        ks = slice(64 * k, 64 * k + 64)
                        nc.gpsimd.tensor_add(h_f_new[:, ks], m1[:, ks],
                                             z2c[k][:])
                    h_f = h_f_new

            # ---- exchange encodings ----
            if USE_RDMA:
                # SBUF->SBUF remote DMA all-broadcast of h repacked to
                # [16, 1024] (2KB/partition: 8x fewer, 8x bigger packets than
                # [128,128]'s 256B lines). Desc-gen (incl. the ~6us Q7 SWDGE
                # lib swap) was emitted before the GRU loop; only the pack,
                # trigger and waits sit after the recurrence. The unpack to
                # [128, 8*128] happens inside the DRAM bounce DMA for free.
                nc.sync.dma_start(
                    pk[:].rearrange("p (g c) -> p g c", g=8, c=128),
                    h_bf[:].rearrange("(g p) c -> p g c", g=8, p=16))
                with tc.tile_critical():
                    nc.gpsimd.bir_kernel_barrier_wait([list(range(NCORES))])
                    nc.gpsimd.wait_ge(psem, NCORES)
                    nc.gpsimd.trigger_dma(count=NCORES)
                    nc.gpsimd.wait_ge(rsem, 16)
                ag_d = dramp.tile([128, NCORES * 128], bf16, tag="agd",
                                  name="agd")
                nc.sync.dma_start(
                    ag_d[:].rearrange("(g p) (j c) -> p j g c",
                                      g=8, p=16, j=NCORES, c=128),
                    ag_p[:].rearrange("p (j g c) -> p j g c",
                                      j=NCORES, g=8, c=128))
                slots = [ag_d[:, 128 * s:128 * s + 128] for s in range(NCORES)]
            else:
                ag_in = dramp.tile([128, 128], bf16, tag="agin", name="agin")
                ag_g = dramp.tile([NCORES, 128, 128], bf16, tag="agg",
                                  name="agg")
                nc.sync.dma_start(ag_in[:, 0:64], h_bf[0][:])
                nc.sync.dma_start(ag_in[:, 64:128], h_bf[1][:])
                nc.gpsimd.collective_compute(
                    "AllGather", mybir.AluOpType.bypass,
                    replica_groups=[list(range(NCORES))],
                    ins=[ag_in.opt()], outs=[ag_g.opt()])
                slots = [ag_g[s] for s in range(NCORES)]

            # per-core q slice: rows [32*co, 32*co+32) live on q-shard co//2
            # whose tile sits in slot F[co ^ co//2] (RDMA) / co//2; half co%2.
            qloc = pp.tile([128, 2 * IBLK], bf16, tag="qloc", name="qloc")  # [p, c*32+b]
            pid = nc.scalar.partition_id()
            for co in range(NCORES):
                sq = F_SLOT[co ^ (co // 2)] if USE_RDMA else co // 2
                src_v = slots[sq].rearrange(
                    "p (c h b) -> p c h b", c=2, h=2, b=32)
                nc.scalar.dma_start(
                    qloc[:].rearrange("p (c b) -> p c b", c=2, b=32),
                    src_v[:, :, co % 2, :], cond=(pid == co))
            # rT in slot order: reply tiles occupy slots 4-7 on q-cores and
            # slots 0-3 on r-cores; the j-block order is undone on the host.
            rT = pp.tile([128, 2 * B], bf16, tag="rT", name="rT")  # [p, c*256 + j]
            if USE_RDMA:
                groups = ((4, nc.gpsimd), (0, nc.scalar))
            else:
                groups = ((4, nc.gpsimd),)
            for sb, qeng in groups:
                cond = None
                if USE_RDMA:
                    qpid = qeng.partition_id()
                    cond = (qpid < 4) if sb == 4 else (qpid >= 4)
                for c in range(2):
                    for s in range(NSH):
                        qeng.dma_start(
                            rT[:, 256 * c + 64 * s:256 * c + 64 * s + 64],
                            slots[sb + s][:, 64 * c:64 * c + 64],
                            cond=cond)

            # ---- classifier ----
            # fused K=4 outer-product operands, one MM per (i-pair, m):
            # lhs4 rows [wd; q1_even; wd; q1_odd], rhs4 rows
            # [dist_even 0; ones 0; 0 dist_odd; 0 ones] per 512-col block
            lhs4 = pp.tile([4, IBLK // 2 * D_HID], bf16, tag="lhs4",
                           name="lhs4")
            nc.sync.dma_start(lhs4[0:1, :], wdt[:])
            nc.sync.dma_start(lhs4[2:3, :], wdt[:])
            rhs4 = pp.tile([4, IBLK * B], bf16, tag="rhs4", name="rhs4")
            nc.sync.dma_start(rhs4[:], rhsb[:])

            r1tb2 = pp.tile([128, 4 * B], f32, tag="r1tb2", name="r1tb2")
            with tc.tile_pool(name="spsum", bufs=2, space="PSUM") as sps:
                # Q1 rows for my i's: [32, 256] bf16
                ps = sps.tile([IBLK, D_HID], f32, tag="sps", name="sps")
                for c in range(2):
                    nc.tensor.matmul(ps[:], qloc[:, 32 * c:32 * c + 32],
                                     w1q_s[c][:], start=(c == 0), stop=(c == 1))
                q1 = pp.tile([IBLK, D_HID], bf16, tag="q1", name="q1")
                nc.scalar.activation(q1[:], ps[:], AF.Copy, bias=0.0)
                nc.sync.dma_start(lhs4[1:2, :], q1[0:16, :])
                nc.sync.dma_start(lhs4[3:4, :], q1[16:32, :])

                # dist rows for my i's: [32, 256] bf16
                ps2 = sps.tile([IBLK, B], f32, tag="sps", name="sps")
                for c in range(2):
                    nc.tensor.matmul(ps2[:], qloc[:, 32 * c:32 * c + 32],
                                     rT[:, 256 * c:256 * c + 256],
                                     start=(c == 0), stop=(c == 1))
                dist = pp.tile([IBLK, B], bf16, tag="dist", name="dist")
                nc.scalar.activation(dist[:], ps2[:], AF.Copy, bias=0.0)
                nc.sync.dma_start(
                    rhs4[0:1, :].rearrange("o (p ii j) -> o p ii j",
                                           p=IBLK // 2, ii=2, j=B)[:, :, 0, :],
                    dist[0:16, :])
                nc.sync.dma_start(
                    rhs4[2:3, :].rearrange("o (p ii j) -> o p ii j",
                                           p=IBLK // 2, ii=2, j=B)[:, :, 1, :],
                    dist[16:32, :])

                # R1T + b1: [128, m*256 + j] f32
                r1tb = pp.tile([128, 2 * B], f32, tag="r1tb", name="r1tb")
                for m in range(2):
                    ps3 = sps.tile([128, B], f32, tag="sps", name="sps")
                    for k in range(2):
                        nc.tensor.matmul(ps3[:],
                                         w1r_s[k][:, 128 * m:128 * m + 128],
                                         rT[:, 256 * k:256 * k + 256],
                                         start=(k == 0), stop=(k == 1))
                    nc.scalar.activation(r1tb[:, 256 * m:256 * m + 256], ps3[:],
                                         AF.Identity, bias=b1_s[:, m:m + 1])

                r2v = r1tb2[:].rearrange("p (m ii j) -> p m ii j", m=2, ii=2,
                                         j=B)
                for ii in range(2):
                    nc.vector.tensor_copy(
                        r2v[:, :, ii, :],
                        r1tb[:].rearrange("p (m j) -> p m j", m=2, j=B))

            with (
                tc.tile_pool(name="hpsum", bufs=2, space="PSUM") as hps,
                tc.tile_pool(name="lpsum", bufs=2, space="PSUM") as lps,
                tc.tile_pool(name="cls", bufs=3) as cp,
            ):
                out_sb = pp.tile([D_OUT, IBLK * B], f32, tag="outsb", name="outsb")
                gelu_af = AF.Tanh if sim_gelu else AF.Gelu_apprx_tanh
                l_ps = None
                for pr in range(IBLK // 2):
                    # h1 pair tile: col = 512*m + 256*ii + j  (ii = i in pair)
                    h_ps = hps.tile([128, 4 * B], f32, tag="hps", name="hps")
                    for m in range(2):
                        nc.tensor.matmul(
                            h_ps[:, 512 * m:512 * m + 512],
                            lhs4[0:4,
                                 D_HID * pr + 128 * m:D_HID * pr + 128 * m + 128],
                            rhs4[0:4, 2 * B * pr:2 * B * pr + 2 * B],
                            start=True, stop=True)
                    h1p = cp.tile([128, 4 * B], f32, tag="h1p", name="h1p")
                    nc.vector.tensor_add(h1p[:], h_ps[:], r1tb2[:])
                    h1 = cp.tile([128, 4 * B], bf16, tag="h1", name="h1")
                    nc.scalar.activation(h1[:], h1p[:], gelu_af)
                    if pr % 2 == 0:
                        l_ps = lps.tile([D_OUT, 4 * B], f32, tag="lps",
                                        name="lps")
                    lsl = slice(512 * (pr % 2), 512 * (pr % 2) + 512)
                    for k in range(2):
                        nc.tensor.matmul(l_ps[:, lsl], w2_s[k][:],
                                         h1[:, 512 * k:512 * k + 512],
                                         start=(k == 0), stop=(k == 1))
                    if pr % 2 == 1:
                        osl = slice(512 * (pr - 1), 512 * (pr - 1) + 1024)
                        if pr % 4 == 1:
                            nc.scalar.activation(out_sb[:, osl], l_ps[:],
                                                 AF.Copy, bias=0.0)
                        else:
                            nc.vector.tensor_copy(out_sb[:, osl], l_ps[:])
                nc.sync.dma_start(out[:], out_sb[:])

    nc.compile()
    return nc


def _rhs_base():
    """[4, IBLK*B] pattern: per 512-col pair-block rows are
    [0,0],[ones,0],[0,0],[0,ones] - dist blocks get DMA'd in on device."""
    r = np.zeros((4, IBLK * B), dtype=BF16)
    v = r.reshape(4, IBLK // 2, 2, B)
    v[1, :, 0, :] = 1.0
    v[3, :, 1, :] = 1.0
    return r


def _prep_inputs(inputs):
    """Host-side prep: embed+transpose sequences, split weights, per-core maps."""
    emb = inputs["embeddings"]
    in_maps = []
    f32 = np.float32

    # classifier tensors (identical on all cores)
    W1, b1, W2 = (inputs["W1"], inputs["b1"], inputs["W2"])
    common = {
        "w1q": np.ascontiguousarray(W1[:H]).astype(BF16),
        "w1r": np.ascontiguousarray(W1[H + 1:]).astype(BF16),
        "wdt": np.tile(np.ascontiguousarray(W1[H:H + 1]).astype(BF16),
                       (1, IBLK // 2)),
        "rhsb": _rhs_base(),
        "b1": b1.astype(f32),
        "w2": W2.astype(BF16),
    }

    for core in range(NCORES):
        enc = core // NSH
        s = core % NSH
        if enc == 0:
            seqs, lens = inputs["input_queries"], inputs["query_lengths"]
            Wg, bgv, Wc, bcv = (inputs["Wg_q"], inputs["bg_q"],
                                inputs["Wc_q"], inputs["bc_q"])
        else:
            seqs, lens = inputs["input_replies"], inputs["reply_lengths"]
            Wg, bgv, Wc, bcv = (inputs["Wg_r"], inputs["bg_r"],
                                inputs["Wc_r"], inputs["bc_r"])
        rows = slice(BSH * s, BSH * s + BSH)
        xe = emb[seqs[rows]]                       # [64, 40, 256]
        xT = np.transpose(xe, (2, 1, 0)).reshape(E, BT)  # col = t*64+b
        lmask = (np.arange(T)[:, None] >= lens[rows][None, :]) \
            .astype(f32).reshape(1, BT)
        ones_row = np.ones((1, BT), f32)
        xembT = np.concatenate([xT, lmask, ones_row], axis=0).astype(BF16)

        mask_row = np.concatenate([np.zeros(H, f32), np.full(H, 30.0, f32)])
        wxg = np.concatenate([Wg[:E], mask_row[None, :], bgv[None, :]],
                             axis=0).astype(BF16)
        wxc = np.concatenate([Wc[:E], np.zeros((1, H), f32), bcv[None, :]],
                             axis=0).astype(BF16)

        m = {
            "xembT": xembT,
            "whg": np.ascontiguousarray(Wg[E:]).astype(BF16),
            "wxg": wxg,
            "wch": np.ascontiguousarray(Wc[E:]).astype(BF16),
            "wxc": wxc,
        }
        m.update(common)
        in_maps.append(m)
    return in_maps


def run_cores(in_maps, trace=False):
    from concourse.bass_utils import run_bass_kernel_spmd
    from concourse.bass_interp import get_hw_module

    if "nc" not in _cache:
        _cache["nc"] = _build()
    nc = _cache["nc"]
    old = nc.m
    nc.m = _cache.setdefault("hwm", get_hw_module(nc.m))
    try:
        res = run_bass_kernel_spmd(nc, in_maps, core_ids=list(range(NCORES)),
                                   trace=trace)
    finally:
        nc.m = old
    return res


def kernel(**inputs):
    in_maps = _prep_inputs(inputs)
    res = run_cores(in_maps)
    logits = np.zeros((B, B, 2), np.float32)
    for core in range(NCORES):
        o = res.results[core]["out"]               # [2, 32*256]
        # pair layout: col = 512*pr + 256*ii + j_local, local row = 16*ii + pr;
        # j_local follows the slot order of rT -> un-permute to global j.
        blk = o.reshape(2, 16, 2, B).transpose(2, 1, 3, 0).reshape(IBLK, B, 2)
        logits[IBLK * core:IBLK * core + IBLK, _jperm(core)] = blk
    logits += inputs["b2"].astype(np.float32)
    pos = logits[np.arange(B), np.arange(B)]
    qi, ri = np.nonzero(~np.eye(B, dtype=bool))
    neg = logits[qi, ri]
    return np.concatenate([pos, neg], axis=0).astype(np.float32)


if __name__ == "__main__":
    _build()
    print("build OK")

